# revision 2
# baseline (speedup 1.0000x reference)
"""MemoryGate kernel for Trainium2 (8 NeuronCores, SPMD) — v2.

Math (per batch b):
    mp   = memory[b] @ W_mem.T                      [M, D]
    S    = hidden[b] @ mp.T / sqrt(D)               [N, M]
    A    = softmax(S, axis=-1)
    ctx  = A @ mp                                   [N, D]
    gate = sigmoid(hidden @ Wg_h.T + ctx @ Wg_c.T + b_gate)
    out  = rmsnorm(hidden + gate * ctx) * norm_w

Sharding: 8 cores = 4 batches x 2 N-halves; BN = 2048 rows per core.

v2 design: fully transposed pipeline in fp8 (e4m3) with DoubleRow matmuls.
All big tensors live in [d/m-partition, n-free] layout so that
  * scoresT = matmul(lhsT=mpT, rhs=hT)            [M, N]  (softmax along
    partitions -> column sums via all-ones matmuls, which also broadcast)
  * ctxT    = matmul(lhsT=mp,  rhs=E)             [D, N]  (E = exp(scoresT),
    unnormalized; the softmax denominator is applied to ctx afterwards)
  * gateT   = matmul(lhsT=WgT, rhs=[hT; ctxT])    [D, N]
so NO on-chip transposes of attention/ctx are needed at all.
The output is produced transposed ([D, BN]); the host transposes back.

Scaling for fp8 range (values quantized in well-scaled form):
    WmT8 = fp8(W_mem.T * S_WM), Wg8 = fp8(W_gate * S_WG pre-permuted)
    mpT8 = (Q/S_WM) * A1psum = Q*mp  with Q = S_WG/sqrt(D)
    scores_psum = Q*(h.mp) = S_WG*scores_true -> exp(ps/S_WG - SHIFT)
    mp8 = A2psum/S_WM = mp;  ctx8 = ctx exactly;  sigmoid(ps/S_WG + b)
The exp SHIFT keeps E below the TRN fp8 max (240); it cancels in the
softmax normalization.

Weight-reuse loop order: every stationary (lhsT) tile is loaded once and
used for 4 moving tiles (the four 512-column n/m chunks), hiding LDWEIGHTS
under the matmul stream on hardware.
"""

import math
import os
import sys

for _p in ("/opt/trn_rl_repo", "/root/.axon_site/_ro/trn_rl_repo"):
    if os.path.isdir(_p) and _p not in sys.path:
        sys.path.append(_p)

import numpy as np

P = 128
MODE = "fp8"          # "fp8" (DoubleRow) or "bf16" fallback
S_WM = 16.0
S_WG = 32.0
EXP_SHIFT = 2.0
EPS = 1e-6


def build_program(mode=MODE, BN=2048, M=2048, D=2048, E=1024):
    import concourse.tile as tile
    from concourse import bacc, mybir

    f32 = mybir.dt.float32
    bf16 = mybir.dt.bfloat16
    AF = mybir.ActivationFunctionType

    DR = mode == "fp8"
    mmdt = mybir.dt.float8e4 if DR else bf16
    PM = mybir.MatmulPerfMode.DoubleRow if DR else None
    ks = 2 if DR else 1

    kD, kE, mT = D // P, E // P, M // P          # 16, 8, 16
    FC = 512
    NBLK = BN // FC                               # 4
    MC = M // FC                                  # 4
    DC = D // FC                                  # 4
    pD, pE, pM = kD // ks, kE // ks, mT // ks     # passes per accumulation
    Q = S_WG / math.sqrt(D)

    nc = bacc.Bacc("TRN2", target_bir_lowering=False, debug=False)

    hT8_d = nc.dram_tensor("hT8", [D, BN], mmdt, kind="ExternalInput")
    hTbf_d = nc.dram_tensor("hTbf", [D, BN], bf16, kind="ExternalInput")
    memT_d = nc.dram_tensor("memT8", [E, M], mmdt, kind="ExternalInput")
    WmT_d = nc.dram_tensor("WmT8", [E, D], mmdt, kind="ExternalInput")
    Wg_d = nc.dram_tensor("Wg8", [mT * P, 2 * D], mmdt, kind="ExternalInput")
    bgT_d = nc.dram_tensor("bgT", [P, kD], f32, kind="ExternalInput")
    nwT_d = nc.dram_tensor("nwT", [P, kD], f32, kind="ExternalInput")
    ones_d = nc.dram_tensor("ones", [P, 2 * P], mmdt, kind="ExternalInput")
    out_d = nc.dram_tensor("out", [D, BN], f32, kind="ExternalOutput")

    with tile.TileContext(nc) as tc:
        with (
            tc.tile_pool(name="const", bufs=1) as const,
            tc.tile_pool(name="hold", bufs=1) as hold,
            tc.tile_pool(name="psA", bufs=8, space="PSUM") as psA,
        ):
            ones_sb = const.tile([P, 2, P], mmdt, tag="ones", name="ones_sb")
            nc.sync.dma_start(ones_sb, ones_d[:])
            bgT_sb = const.tile([P, kD], f32, tag="bg", name="bgT_sb")
            nc.sync.dma_start(bgT_sb, bgT_d[:])
            nwT_sb = const.tile([P, kD], f32, tag="nw", name="nwT_sb")
            nc.sync.dma_start(nwT_sb, nwT_d[:])
            eps_sb = const.tile([P, 1], f32, tag="eps", name="eps_sb")
            nc.vector.memset(eps_sb, EPS)
            shift_sb = const.tile([P, 1], f32, tag="shift", name="shift_sb")
            nc.vector.memset(shift_sb, -EXP_SHIFT)

            # whole-kernel residents
            hT8 = hold.tile([P, kD, BN], mmdt, tag="hT8", name="hT8_sb")
            mpT8 = hold.tile([P, kD, M], mmdt, tag="big1", name="mpT8_sb")
            ctx8 = hold.tile([P, kD, BN], mmdt, tag="ctx8", name="ctx8_sb")
            recip = hold.tile([P, NBLK, FC], f32, tag="recip", name="recip_sb")

            # pool for tensors dead after phase C (mp8, E)
            scmp_cm = tc.tile_pool(name="scmp", bufs=1)
            scmp = scmp_cm.__enter__()
            mp8 = scmp.tile([P, mT, D], mmdt, tag="mp8", name="mp8_sb")
            E_sb = scmp.tile([P, mT, BN], mmdt, tag="E", name="E_sb")

            # ---------------- Stage A: mpT8 and mp8 --------------------
            with tc.tile_pool(name="ain", bufs=1) as ain:
                memT_sb = ain.tile([P, kE, M], mmdt, tag="memT", name="memT_sb")
                WmT_sb = ain.tile([P, kE, D], mmdt, tag="WmT", name="WmT_sb")
                for k in range(kE):
                    nc.sync.dma_start(memT_sb[:, k, :], memT_d[k * P:(k + 1) * P, :])
                    nc.sync.dma_start(WmT_sb[:, k, :], WmT_d[k * P:(k + 1) * P, :])
                for k in range(kD):
                    nc.sync.dma_start(hT8[:, k, :], hT8_d[k * P:(k + 1) * P, :])

                # A1: mpT8[d, m] = Q/S_WM * sum_e WmT8[e, d] * memT8[e, m]
                for dt in range(kD):
                    pss = [psA.tile([P, FC], f32, tag="ps", name=f"a1_{dt}_{m}")
                           for m in range(MC)]
                    for e in range(pE):
                        w = WmT_sb[:, e * ks:(e + 1) * ks, dt * P:(dt + 1) * P]
                        for m in range(MC):
                            nc.tensor.matmul(
                                pss[m], w,
                                memT_sb[:, e * ks:(e + 1) * ks, m * FC:(m + 1) * FC],
                                start=(e == 0), stop=(e == pE - 1), perf_mode=PM)
                    for m in range(MC):
                        nc.scalar.mul(mpT8[:, dt, m * FC:(m + 1) * FC], pss[m],
                                      Q / S_WM)
                # A2: mp8[m, d] = 1/S_WM * sum_e memT8[e, m] * WmT8[e, d]
                for mt in range(mT):
                    pss = [psA.tile([P, FC], f32, tag="ps", name=f"a2_{mt}_{d}")
                           for d in range(DC)]
                    for e in range(pE):
                        w = memT_sb[:, e * ks:(e + 1) * ks, mt * P:(mt + 1) * P]
                        for d in range(DC):
                            nc.tensor.matmul(
                                pss[d], w,
                                WmT_sb[:, e * ks:(e + 1) * ks, d * FC:(d + 1) * FC],
                                start=(e == 0), stop=(e == pE - 1), perf_mode=PM)
                    for d in range(DC):
                        nc.scalar.mul(mp8[:, mt, d * FC:(d + 1) * FC], pss[d],
                                      1.0 / S_WM)

            # ---------------- S: E = exp(scoresT) ; colsums ------------
            if True:
                for mt in range(mT):
                    pss = [psA.tile([P, FC], f32, tag="ps", name=f"s_{mt}_{b}")
                           for b in range(NBLK)]
                    for k in range(pD):
                        w = mpT8[:, k * ks:(k + 1) * ks, mt * P:(mt + 1) * P]
                        for b in range(NBLK):
                            nc.tensor.matmul(
                                pss[b], w,
                                hT8[:, k * ks:(k + 1) * ks, b * FC:(b + 1) * FC],
                                start=(k == 0), stop=(k == pD - 1), perf_mode=PM)
                    for b in range(NBLK):
                        nc.scalar.activation(
                            E_sb[:, mt, b * FC:(b + 1) * FC], pss[b], AF.Exp,
                            bias=shift_sb, scale=1.0 / S_WG)

                # column sums of E (broadcast to all 128 partitions via
                # all-ones stationary), then reciprocal
                for b in range(NBLK):
                    cs = psA.tile([P, FC], f32, tag="ps", name=f"cs_{b}")
                    for j in range(pM):
                        nc.tensor.matmul(
                            cs, ones_sb[:, :ks, :],
                            E_sb[:, j * ks:(j + 1) * ks, b * FC:(b + 1) * FC],
                            start=(j == 0), stop=(j == pM - 1), perf_mode=PM)
                    nc.vector.reciprocal(recip[:, b, :], cs)

                # ------------ C: ctx8 = (mp8^T E) * recip --------------
                for dt in range(kD):
                    pss = [psA.tile([P, FC], f32, tag="ps", name=f"c_{dt}_{b}")
                           for b in range(NBLK)]
                    for j in range(pM):
                        w = mp8[:, j * ks:(j + 1) * ks, dt * P:(dt + 1) * P]
                        for b in range(NBLK):
                            nc.tensor.matmul(
                                pss[b], w,
                                E_sb[:, j * ks:(j + 1) * ks, b * FC:(b + 1) * FC],
                                start=(j == 0), stop=(j == pM - 1), perf_mode=PM)
                    for b in range(NBLK):
                        nc.vector.tensor_mul(
                            ctx8[:, dt, b * FC:(b + 1) * FC], pss[b],
                            recip[:, b, :])

            scmp_cm.__exit__(None, None, None)

            # ---------------- G: gateT ---------------------------------
            gate8 = hold.tile([P, kD, BN], mmdt, tag="big1", name="gate8_sb")
            with tc.tile_pool(name="wg", bufs=3) as wg:
                for t in range(kD):
                    Wt = wg.tile([P, 2 * kD, P], mmdt, tag="wt", name=f"wt{t}")
                    nc.sync.dma_start(Wt, Wg_d[t * P:(t + 1) * P, :])
                    pss = [psA.tile([P, FC], f32, tag="ps", name=f"g_{t}_{b}")
                           for b in range(NBLK)]
                    for j in range(2 * pD):
                        rhs = hT8 if j < pD else ctx8
                        jj = j % pD
                        for b in range(NBLK):
                            nc.tensor.matmul(
                                pss[b], Wt[:, j * ks:(j + 1) * ks, :],
                                rhs[:, jj * ks:(jj + 1) * ks, b * FC:(b + 1) * FC],
                                start=(j == 0), stop=(j == 2 * pD - 1),
                                perf_mode=PM)
                    for b in range(NBLK):
                        nc.scalar.activation(
                            gate8[:, t, b * FC:(b + 1) * FC], pss[b], AF.Sigmoid,
                            bias=bgT_sb[:, t:t + 1], scale=1.0 / S_WG)

            # ---------------- F: fused + rmsnorm -----------------------
            with (
                tc.tile_pool(name="ff", bufs=2) as ff,
                tc.tile_pool(name="fh", bufs=5) as fh,
                tc.tile_pool(name="fo", bufs=3) as fo,
                tc.tile_pool(name="fr", bufs=1) as fr,
            ):
                for b in range(NBLK):
                    n0 = b * FC
                    fused = ff.tile([P, kD, FC], f32, tag="fused", name=f"fu{b}")
                    sq = ff.tile([P, kD, FC], mmdt, tag="sq", name=f"sq{b}")
                    for dt in range(kD):
                        hc = fh.tile([P, FC], bf16, tag="hc", name=f"hc{b}_{dt}")
                        nc.sync.dma_start(
                            hc, hTbf_d[dt * P:(dt + 1) * P, n0:n0 + FC])
                        tm = fh.tile([P, FC], bf16, tag="tm", name=f"tm{b}_{dt}")
                        nc.vector.tensor_mul(
                            tm, gate8[:, dt, n0:n0 + FC], ctx8[:, dt, n0:n0 + FC])
                        nc.vector.tensor_add(fused[:, dt, :], tm, hc)
                        nc.scalar.activation(sq[:, dt, :], fused[:, dt, :],
                                             AF.Square)
                    ssq = psA.tile([P, FC], f32, tag="ps", name=f"ssq{b}")
                    for j in range(pD):
                        nc.tensor.matmul(
                            ssq, ones_sb[:, :ks, :],
                            sq[:, j * ks:(j + 1) * ks, :],
                            start=(j == 0), stop=(j == pD - 1), perf_mode=PM)
                    rstd = fr.tile([P, FC], f32, tag="rstd", name=f"rstd{b}")
                    nc.scalar.activation(rstd, ssq, AF.Sqrt, bias=eps_sb,
                                         scale=1.0 / D)
                    rinv = fr.tile([P, FC], f32, tag="rinv", name=f"rinv{b}")
                    nc.vector.reciprocal(rinv, rstd)
                    for dt in range(kD):
                        ot = fo.tile([P, FC], f32, tag="ot", name=f"ot{b}_{dt}")
                        nc.vector.tensor_mul(ot, fused[:, dt, :], rinv)
                        nc.scalar.mul(ot, ot, nwT_sb[:, dt:dt + 1])
                        nc.gpsimd.dma_start(
                            out_d[dt * P:(dt + 1) * P, n0:n0 + FC], ot)

    nc.compile()
    return nc


_PROG_CACHE = {}


def _get_program(key, **kw):
    if key not in _PROG_CACHE:
        _PROG_CACHE[key] = build_program(**kw)
    return _PROG_CACHE[key]


def _prep_shared(W_mem, W_gate, b_gate, norm_w, mode):
    import ml_dtypes
    qdt = ml_dtypes.float8_e4m3fn if mode == "fp8" else ml_dtypes.bfloat16
    D = W_mem.shape[0]
    kD = D // P
    WmT = np.ascontiguousarray(W_mem.T * S_WM).astype(qdt)
    # Wg_pre[t, p, q, m] = W_gate[128t+m, 128q+p] * S_WG  -> [mT*P, 2D]
    W4 = (W_gate * S_WG).reshape(kD, P, 2 * kD, P)
    Wg = np.ascontiguousarray(W4.transpose(0, 3, 2, 1)).reshape(kD * P, -1)
    Wg = np.ascontiguousarray(Wg).astype(qdt)
    bgT = np.ascontiguousarray(b_gate.reshape(kD, P).T, dtype=np.float32)
    nwT = np.ascontiguousarray(norm_w.reshape(kD, P).T, dtype=np.float32)
    ones = np.ones((P, 2 * P), dtype=np.float32).astype(qdt)
    return WmT, Wg, bgT, nwT, ones


def kernel(hidden_states, memory, W_mem, W_gate, b_gate, norm_w):
    from concourse.bass_utils import run_bass_kernel_spmd
    import ml_dtypes

    mode = MODE
    qdt = ml_dtypes.float8_e4m3fn if mode == "fp8" else ml_dtypes.bfloat16
    B, N, D = hidden_states.shape
    _, M, E = memory.shape
    NC = 8
    H = NC // B                      # N-splits per batch (2)
    BN = N // H                      # rows per core (2048)

    prog = _get_program((mode, BN, M, D, E), mode=mode, BN=BN, M=M, D=D, E=E)

    WmT, Wg, bgT, nwT, ones = _prep_shared(W_mem, W_gate, b_gate, norm_w, mode)

    in_maps = []
    for c in range(NC):
        b, h = c // H, c % H
        hsT = np.ascontiguousarray(
            hidden_states[b, h * BN:(h + 1) * BN, :].T)
        in_maps.append({
            "hT8": hsT.astype(qdt),
            "hTbf": hsT.astype(ml_dtypes.bfloat16),
            "memT8": np.ascontiguousarray(memory[b].T).astype(qdt),
            "WmT8": WmT, "Wg8": Wg, "bgT": bgT, "nwT": nwT, "ones": ones,
        })

    res = run_bass_kernel_spmd(prog, in_maps, core_ids=list(range(NC)))
    out = np.empty((B, N, D), dtype=np.float32)
    for c in range(NC):
        b, h = c // H, c % H
        out[b, h * BN:(h + 1) * BN, :] = res.results[c]["out"].T
    return out
